# revision 14
# baseline (speedup 1.0000x reference)
"""BalancedBatchNorm2d Trainium2 kernel (v2: bf16 stores + balanced stats).

Math: the reference's per-class segment-sum collapses algebraically:
  mean[c]  = (1/(L*HW)) * sum_b w_b * sum_hw X[b,c,:,:],  w_b = 1/count(label_b)
  var[c]   = E[X^2] - 2*mean*E[X] + mean^2   (plain moments over (B,HW))
  Y        = X*scale[c] + bias[c],  scale = gamma/sqrt(var+eps), bias = beta - mean*scale

Sharding: channels across the 8 cores (8 ch/core) -> every core owns all
batches for its channels, so all reductions are core-local (no collectives).

Precision plan (rel-err budget is 2e-2; host sim of this plan gives 3.9e-3):
  - mean from exact f32 sums (DVE tensor_scalar Copy with accum_out; the
    bf16 downcast happens on the write port, accumulate is the fp32 ALU)
  - sumsq partly from the bf16 copy (zero-mean quantization error averages
    out over 512K samples/channel)
  - normalize reads the f32 copy, writes bf16 (output quantization only)
  - Y stored as bf16 -> store HBM traffic halves (16.8 -> 8.4 MB/core)

Per-core layout: x[NT=32, 128, HW=1024] f32, partition p = b_lo*8 + ch
(16 batches x 8 channels per tile). Engine plan per tile during load:
  SP   : 33 tile loads -> (after normalize) 11 chunked bf16 stores
  DVE  : fused convert+sum (x_sb f32 -> x16 bf16, accum_out = row sum);
         sumsq via STT on x16 for tiles t%4==3 + the split last tile;
         stats chunks, finalize algebra, normalize (tiles shared with ACT)
  ACT  : consts load, sumsq via Square+accum from f32 for the other tiles,
         the one sqrt, normalize for the back half of each store chunk
  PE   : 3 accumulated [128x128]@[128,3] matmuls vs the channel-group
         selector: cross-partition per-channel sums, broadcast to all rows.
"""

import numpy as np

import concourse.bass as bass
from concourse import mybir
from concourse.bass_utils import run_bass_kernel_spmd

B, C, H, W = 512, 64, 32, 32
HW = H * W
L = 100  # num classes
EPS = 1e-6
NCORES = 8
CPC = C // NCORES  # channels per core = 8
BPT = 128 // CPC  # batches per tile = 16
NT = B // BPT  # tiles per core = 32
F32 = mybir.dt.float32
BF16 = mybir.dt.bfloat16

# consts tensor column layout (wmat has NT+1 cols: last = dup of col NT-1,
# covering the second half of the split final tile)
NWM = NT + 1
NCOL = NWM + 128 + 3
COL_RSEL = NWM
COL_G = NWM + 128
COL_B = NWM + 129
COL_EPS = NWM + 130

_NC_CACHE = {}

# sumsq ownership: ACT handles most tiles from the f32 copy, DVE handles
# every 4th + the split last tile from the bf16 copy (2x DVE rate)
DVE_SQ = set(range(3, NT - 1, 4))  # {3,7,11,15,19,23,27}
ACT_SQ = [t for t in range(NT - 1) if t not in DVE_SQ]  # 24 tiles

# stats chunks (columns of rs/sq): A = 0..27, B = 28..31 (incl first half of
# tile 31 as col 31), C = col 32 (second half of tile 31)
CHA = slice(0, 28)
CHB = slice(28, 32)
CHC = slice(32, 33)
ACT_SQ_A = sum(1 for t in ACT_SQ if t < 28)  # 21
ACT_SQ_ALL = len(ACT_SQ)  # 24

# normalize ownership per store chunk: DVE takes the front half, ACT the back
STORE_CHUNKS = [(0, "a"), (0, "b"), (1, 1), (2, 3), (4, 7), (8, 11), (12, 15), (16, 19), (20, 23), (24, 27), (28, 31)]
DVE_NORM = [(0, "a"), (0, "b"), (1, None), (2, None)] + [
    (g, None) for g0 in range(4, NT, 4) for g in (g0, g0 + 1)
]
ACT_NORM = [(3, None)] + [
    (g, None) for g0 in range(4, NT, 4) for g in (g0 + 2, g0 + 3)
]


def _bcast0(col_ap, n):
    # [128,1] column AP -> [128,n] write AP with free-dim stride 0 (sink).
    return bass.AP(
        tensor=col_ap.tensor,
        offset=col_ap.offset,
        ap=[list(col_ap.ap[0]), [0, n]],
    )


def build_nc():
    nc = bass.Bass()
    x_d = nc.declare_dram_parameter("x", [NT, 128, HW], F32, isOutput=False)
    c_d = nc.declare_dram_parameter("consts", [128, NCOL], F32, isOutput=False)
    y_d = nc.declare_dram_parameter("y", [NT, 128, HW], BF16, isOutput=True)

    from contextlib import ExitStack

    c1 = 1.0 / (L * HW)  # balanced-mean scale
    c2 = 1.0 / (B * HW)  # plain-moment scale

    # store-chunk wait thresholds on the two normalize counters
    chunk_waits = []
    di = ai = 0
    for ch in STORE_CHUNKS:
        if ch[1] in ("a", "b"):
            cover = [(0, ch[1])]
        else:
            cover = [(t, None) for t in range(ch[0], ch[1] + 1)]
        for item in cover:
            if item in DVE_NORM:
                di = max(di, DVE_NORM.index(item) + 1)
            if item in ACT_NORM:
                ai = max(ai, ACT_NORM.index(item) + 1)
        chunk_waits.append((di, ai))

    with ExitStack() as ctx:
        small_sem = ctx.enter_context(nc.semaphore("small_sem"))
        load_x = [ctx.enter_context(nc.semaphore(f"load_x{t}")) for t in range(NT - 1)]
        load_la = ctx.enter_context(nc.semaphore("load_la"))
        load_lb = ctx.enter_context(nc.semaphore("load_lb"))
        store_sem = ctx.enter_context(nc.semaphore("store_sem"))
        s_sq_act = ctx.enter_context(nc.semaphore("s_sq_act"))
        s_t3a = ctx.enter_context(nc.semaphore("s_t3a"))
        s_t3b = ctx.enter_context(nc.semaphore("s_t3b"))
        s_t3c = ctx.enter_context(nc.semaphore("s_t3c"))
        s_pe = ctx.enter_context(nc.semaphore("s_pe"))
        s_var = ctx.enter_context(nc.semaphore("s_var"))
        s_sd = ctx.enter_context(nc.semaphore("s_sd"))
        s_nb2 = ctx.enter_context(nc.semaphore("s_nb2"))
        s_nd = ctx.enter_context(nc.semaphore("s_nd"))
        s_na = ctx.enter_context(nc.semaphore("s_na"))
        dvq = ctx.enter_context(nc.semaphore("dvq"))
        x_sb = ctx.enter_context(nc.sbuf_tensor("x_sb", [128, NT, HW], F32))
        x16 = ctx.enter_context(nc.sbuf_tensor("x16", [128, NT, HW], BF16))
        c_sb = ctx.enter_context(nc.sbuf_tensor("c_sb", [128, NCOL], F32))
        rs_col = ctx.enter_context(nc.sbuf_tensor("rs_col", [128, NWM], F32))
        sq_col = ctx.enter_context(nc.sbuf_tensor("sq_col", [128, NWM], F32))
        t3 = ctx.enter_context(nc.sbuf_tensor("t3", [128, 4], F32))
        t4 = ctx.enter_context(nc.sbuf_tensor("t4", [128, 4], F32))
        t5 = ctx.enter_context(nc.sbuf_tensor("t5", [128, 4], F32))
        junk_act = ctx.enter_context(nc.sbuf_tensor("junk_act", [128, 4], F32))
        junk_dve = ctx.enter_context(nc.sbuf_tensor("junk_dve", [128, 4], F32))
        a_t = ctx.enter_context(nc.sbuf_tensor("a_t", [128, 1], F32))
        mean_s = ctx.enter_context(nc.sbuf_tensor("mean_s", [128, 1], F32))
        nvar_t = ctx.enter_context(nc.sbuf_tensor("nvar_t", [128, 1], F32))
        sd_t = ctx.enter_context(nc.sbuf_tensor("sd_t", [128, 1], F32))
        scale_t = ctx.enter_context(nc.sbuf_tensor("scale_t", [128, 1], F32))
        nbias_t = ctx.enter_context(nc.sbuf_tensor("nbias_t", [128, 1], F32))
        nbias2_t = ctx.enter_context(nc.sbuf_tensor("nbias2_t", [128, 1], F32))
        p3 = ctx.enter_context(nc.psum_tensor("p3", [128, 4], F32))

        wm_ap = c_sb[:, 0:NWM]
        rsel_ap = c_sb[:, COL_RSEL : COL_RSEL + 128]
        gsc_ap = c_sb[:, COL_G : COL_G + 1]  # -1/gamma^2
        bv_ap = c_sb[:, COL_B : COL_B + 1]
        epsg_ap = c_sb[:, COL_EPS : COL_EPS + 1]  # eps/gamma^2

        def norm_src_dst(t, h):
            if h == "a":
                return x_sb[:, t, 0 : HW // 2], x16[:, t, 0 : HW // 2]
            if h == "b":
                return x_sb[:, t, HW // 2 : HW], x16[:, t, HW // 2 : HW]
            return x_sb[:, t, :], x16[:, t, :]

        with nc.Block() as block:

            @block.sync
            def _(sp):
                for t in range(NT - 1):
                    sp.dma_start(out=x_sb[:, t, :], in_=x_d[t]).then_inc(load_x[t], 16)
                t = NT - 1
                sp.dma_start(
                    out=x_sb[:, t, 0 : HW // 2], in_=x_d[t][:, 0 : HW // 2]
                ).then_inc(load_la, 16)
                sp.dma_start(
                    out=x_sb[:, t, HW // 2 : HW], in_=x_d[t][:, HW // 2 : HW]
                ).then_inc(load_lb, 16)
                n_dma = 0
                for ch, (dw, aw) in zip(STORE_CHUNKS, chunk_waits):
                    sp.wait_ge(s_nd, dw)
                    if aw:
                        sp.wait_ge(s_na, aw)
                    if ch[1] == "a":
                        dst = y_d[0][:, 0 : HW // 2]
                        src = x16[:, 0, 0 : HW // 2]
                    elif ch[1] == "b":
                        dst = y_d[0][:, HW // 2 : HW]
                        src = x16[:, 0, HW // 2 : HW]
                    else:
                        g0, g1 = ch
                        k = g1 - g0 + 1
                        dst = bass.AP(
                            tensor=y_d,
                            offset=g0 * 128 * HW,
                            ap=[[HW, 128], [128 * HW, k], [1, HW]],
                        )
                        src = x16[:, g0 : g0 + k, :]
                    sp.dma_start(out=dst, in_=src).then_inc(store_sem, 16)
                    n_dma += 1
                sp.wait_ge(store_sem, 16 * n_dma)

            @block.scalar
            def _(act):
                act.dma_start(out=c_sb[:, :], in_=c_d[:, :]).then_inc(small_sem, 16)
                # dummy op: pull the ACT table load off the critical path
                act.activation(
                    out=junk_act[:, 0:1],
                    in_=junk_act[:, 0:1],
                    func=mybir.ActivationFunctionType.Copy,
                )
                for t in ACT_SQ:
                    act.wait_ge(load_x[t], 16)
                    act.activation(
                        out=_bcast0(junk_act[:, 1:2], HW),
                        in_=x_sb[:, t, :],
                        func=mybir.ActivationFunctionType.Square,
                        accum_out=sq_col[:, t : t + 1],
                    ).then_inc(s_sq_act, 1)
                # sd = sqrt(nvar*(-1/g^2) + eps/g^2) = sqrt(var+eps)/|gamma|
                act.wait_ge(small_sem, 16)
                act.wait_ge(s_var, 1)
                act.activation(
                    out=sd_t[:, :],
                    in_=nvar_t[:, :],
                    func=mybir.ActivationFunctionType.Sqrt,
                    scale=gsc_ap,
                    bias=epsg_ap,
                ).then_inc(s_sd, 1)
                # normalize (back half of each store chunk): y = x*scale + (-nbias)
                act.wait_ge(s_nb2, 1)
                for t, h in ACT_NORM:
                    src, dst = norm_src_dst(t, h)
                    act.activation(
                        out=dst,
                        in_=src,
                        func=mybir.ActivationFunctionType.Identity,
                        scale=scale_t[:, :],
                        bias=nbias2_t[:, :],
                    ).then_inc(s_na, 1)

            @block.vector
            def _(dve):
                def _cvtsum(t, f0, f1, col):
                    # x16 = bf16(x), accum_out = exact f32 row-sum
                    dve.tensor_scalar(
                        out=x16[:, t, f0:f1],
                        in0=x_sb[:, t, f0:f1],
                        scalar1=1.0,
                        scalar2=0.0,
                        op0=mybir.AluOpType.mult,
                        op1=mybir.AluOpType.add,
                        accum_out=rs_col[:, col : col + 1],
                    )

                def _sumsq16(t, f0, f1, col):
                    dve.scalar_tensor_tensor(
                        out=_bcast0(junk_dve[:, 0:1], f1 - f0),
                        in0=x16[:, t, f0:f1],
                        scalar=1.0,
                        in1=x16[:, t, f0:f1],
                        op0=mybir.AluOpType.mult,
                        op1=mybir.AluOpType.mult,
                        accum_out=sq_col[:, col : col + 1],
                    )

                def _t3cols(dst, sem, sl, n):
                    # pre-scaled stats columns so the matmul lands finished
                    # moments: P0=mean, P1=2*E[x], P2=E[x^2]
                    dve.scalar_tensor_tensor(
                        out=_bcast0(junk_dve[:, 1:2], n),
                        in0=rs_col[:, sl],
                        scalar=c1,
                        in1=wm_ap[:, sl],
                        op0=mybir.AluOpType.mult,
                        op1=mybir.AluOpType.mult,
                        accum_out=dst[:, 0:1],
                    ).then_inc(sem, 1)
                    dve.tensor_scalar(
                        out=_bcast0(junk_dve[:, 2:3], n),
                        in0=rs_col[:, sl],
                        scalar1=2.0 * c2,
                        scalar2=0.0,
                        op0=mybir.AluOpType.mult,
                        op1=mybir.AluOpType.add,
                        accum_out=dst[:, 1:2],
                    ).then_inc(sem, 1)
                    dve.tensor_scalar(
                        out=_bcast0(junk_dve[:, 3:4], n),
                        in0=sq_col[:, sl],
                        scalar1=c2,
                        scalar2=0.0,
                        op0=mybir.AluOpType.mult,
                        op1=mybir.AluOpType.add,
                        accum_out=dst[:, 2:3],
                    ).then_inc(sem, 1)

                for t in range(28):
                    dve.wait_ge(load_x[t], 16)
                    _cvtsum(t, 0, HW, t)
                    if t in DVE_SQ:
                        _sumsq16(t, 0, HW, t)
                # chunk A stats hide under the load tail
                dve.wait_ge(s_sq_act, ACT_SQ_A)
                dve.wait_ge(small_sem, 16)
                _t3cols(t3, s_t3a, CHA, 28)
                for t in range(28, NT - 1):
                    dve.wait_ge(load_x[t], 16)
                    _cvtsum(t, 0, HW, t)
                    if t in DVE_SQ:
                        _sumsq16(t, 0, HW, t)
                t = NT - 1
                dve.wait_ge(load_la, 16)
                _cvtsum(t, 0, HW // 2, t)
                _sumsq16(t, 0, HW // 2, t)
                dve.wait_ge(s_sq_act, ACT_SQ_ALL)
                _t3cols(t4, s_t3b, CHB, 4)
                dve.wait_ge(load_lb, 16)
                _cvtsum(t, HW // 2, HW, NT)
                _sumsq16(t, HW // 2, HW, NT)
                _t3cols(t5, s_t3c, CHC, 1)
                # finalize algebra straight off PSUM:
                #   mean = P0; a = P1 - mean = 2E[x] - mean; nvar = a*mean - P2
                dve.wait_ge(s_pe, 3)
                dve.tensor_scalar_mul(mean_s[:, :], p3[:, 0:1], 1.0).then_inc(dvq, 1)
                dve.wait_ge(dvq, 1)
                dve.scalar_tensor_tensor(
                    out=a_t[:, :],
                    in0=p3[:, 1:2],
                    scalar=1.0,
                    in1=mean_s[:, :],
                    op0=mybir.AluOpType.mult,
                    op1=mybir.AluOpType.subtract,
                ).then_inc(dvq, 1)
                dve.wait_ge(dvq, 2)
                dve.scalar_tensor_tensor(
                    out=nvar_t[:, :],
                    in0=a_t[:, :],
                    scalar=mean_s[:, :],
                    in1=p3[:, 2:3],
                    op0=mybir.AluOpType.mult,
                    op1=mybir.AluOpType.subtract,
                ).then_inc(s_var, 1)
                # rstd = 1/sd; scale = gamma*rstd; nbias = mean*scale - beta
                dve.wait_ge(s_sd, 1)
                dve.reciprocal(scale_t[:, :], sd_t[:, :]).then_inc(dvq, 1)
                dve.wait_ge(dvq, 3)
                dve.scalar_tensor_tensor(
                    out=nbias_t[:, :],
                    in0=scale_t[:, :],
                    scalar=mean_s[:, :],
                    in1=bv_ap,
                    op0=mybir.AluOpType.mult,
                    op1=mybir.AluOpType.subtract,
                ).then_inc(dvq, 1)
                dve.wait_ge(dvq, 4)

                # y = x*scale - nbias; f32 in, bf16 out into the x16 buffer
                def _norm(t, h):
                    src, dst = norm_src_dst(t, h)
                    dve.tensor_scalar(
                        out=dst,
                        in0=src,
                        scalar1=scale_t[:, :],
                        scalar2=nbias_t[:, :],
                        op0=mybir.AluOpType.mult,
                        op1=mybir.AluOpType.subtract,
                    ).then_inc(s_nd, 1)

                _norm(0, "a")
                _norm(0, "b")
                # nbias2 = -nbias for ACT's Copy(scale*x + bias) form
                dve.tensor_scalar(
                    out=nbias2_t[:, :],
                    in0=nbias_t[:, :],
                    scalar1=-1.0,
                    scalar2=0.0,
                    op0=mybir.AluOpType.mult,
                    op1=mybir.AluOpType.add,
                ).then_inc(s_nb2, 1)
                for t, h in DVE_NORM[2:]:
                    _norm(t, h)

            @block.tensor
            def _(pe):
                pe.wait_ge(small_sem, 16)
                pe.wait_ge(s_t3a, 3)
                pe.matmul(p3[:, 0:3], rsel_ap, t3[:, 0:3], start=True, stop=False).then_inc(s_pe, 1)
                pe.wait_ge(s_t3b, 3)
                pe.matmul(p3[:, 0:3], rsel_ap, t4[:, 0:3], start=False, stop=False).then_inc(s_pe, 1)
                pe.wait_ge(s_t3c, 3)
                pe.matmul(p3[:, 0:3], rsel_ap, t5[:, 0:3], start=False, stop=True).then_inc(s_pe, 1)

    return nc


def get_nc():
    if "nc" not in _NC_CACHE:
        _NC_CACHE["nc"] = build_nc()
    return _NC_CACHE["nc"]


def make_in_maps(X, label, gamma, beta):
    """Host-side sharding: full inputs -> per-core input maps."""
    X = np.asarray(X, dtype=np.float32)
    label = np.asarray(label).astype(np.int64).ravel()
    gamma = np.asarray(gamma, dtype=np.float32).reshape(C)
    beta = np.asarray(beta, dtype=np.float32).reshape(C)

    cnt = np.bincount(label, minlength=L).astype(np.float32)
    cnt = np.maximum(cnt, 1.0)  # absent classes never indexed; avoid div0
    w = (1.0 / cnt[label]).astype(np.float32)  # (B,)

    # wmat[p, t] = w[t*BPT + p // CPC]
    wmat = np.broadcast_to(w.reshape(NT, BPT, 1), (NT, BPT, CPC)).reshape(NT, 128).T
    pch = np.arange(128) % CPC
    rsel = (pch[:, None] == pch[None, :]).astype(np.float32)

    Xr = X.reshape(B, C, HW)
    in_maps = []
    for i in range(NCORES):
        sl = slice(i * CPC, (i + 1) * CPC)
        xs = np.ascontiguousarray(Xr[:, sl, :]).reshape(NT, 128, HW)
        consts = np.empty((128, NCOL), np.float32)
        consts[:, 0:NT] = wmat
        consts[:, NT] = wmat[:, NT - 1]
        consts[:, COL_RSEL : COL_RSEL + 128] = rsel
        g = np.tile(gamma[sl], BPT).astype(np.float64)
        gsq = np.maximum(g * g, 1e-30)
        consts[:, COL_G] = (-1.0 / gsq).astype(np.float32)
        consts[:, COL_B] = np.tile(beta[sl], BPT)
        consts[:, COL_EPS] = (EPS / gsq).astype(np.float32)
        in_maps.append({"x": xs, "consts": consts})
    return in_maps


def assemble_output(results):
    Y = np.empty((B, C, HW), np.float32)
    for i in range(NCORES):
        Y[:, i * CPC : (i + 1) * CPC, :] = (
            results[i]["y"].astype(np.float32).reshape(B, CPC, HW)
        )
    return Y.reshape(B, C, H, W)


def kernel(X, label, gamma, beta):
    in_maps = make_in_maps(X, label, gamma, beta)
    nc = get_nc()
    res = run_bass_kernel_spmd(nc, in_maps, list(range(NCORES)))
    return assemble_output(res.results)
